# revision 29
# baseline (speedup 1.0000x reference)
"""Trainium2 Bass kernel for nn_BaselinePhasorBlock (B=2, L=1024, D=512, K=64).

Algorithm: the phasor-memory cumsum collapses to causal attention
    A[t,s] = cosQ[t]·cosK[s] + sinQ[t]·sinK[s]   (dot over k)
    retrieved = tril(A) @ value
with value = x@Wv + bv reassociated as
    retrieved = (tril(A) @ x) @ Wv + rowsum(tril(A)) ⊗ bv
so the full-length value GEMM (L×D×D) is replaced by a 256-row
post-multiply; tril(A)@x reuses the same masked-score matmuls.

LayerNorm folding (exact), as before:
    LN(retrieved/norm) @ Wo + bo + x
  = scale_t * (r @ Wg - mu_t * cw) + x + (ln_b@Wo + bo)
with Wg = diag(ln_g)@Wo, cw = colsums(Wg), scale_t = 1/sqrt(var_r + eps*norm_t^2).

Sharding (8 cores, SPMD, no collectives): core c -> batch b = c//4, strip pair
i = c%4 owning t-strips [i*128,(i+1)*128) and [(7-i)*128,(8-i)*128).
s-chunks are processed in a PER-CORE PERMUTED slot order (own strips first),
carried entirely by host-packed data (xT columns, xnat chunks, spidx), so the
instruction stream is identical across cores. The causal mask is generated
on-device from tglob/spidx via one fused scalar_tensor_tensor per chunk
(is_ge then mult) -- no mask DMA.

DMA: split across three queues (sync HWDGE / scalar HWDGE / gpsimd SWDGE),
chunked so the key MLP can start as soon as the first wk1/xT chunks land.
"""

import math
from contextlib import ExitStack

import numpy as np

B, L, D, K = 2, 1024, 512, 64
PI = math.pi
NCORES = 8
NSC = L // 128  # 8 s-chunk slots
NDC = D // 128  # 4 d-chunks
EPS = 1e-5

# columns in the packed f32 const tile [128, 12]
CF_BK1, CF_BQ1, CF_BK2D, CF_BQ2D, CF_EPSN2 = 0, 4, 8, 9, 10
CF_W = 12

_CACHE = {}


def _build_program(gelu_override=None):
    import concourse.bacc as bacc
    import concourse.mybir as mybir
    import concourse.tile as tile

    AF = mybir.ActivationFunctionType
    ALU = mybir.AluOpType
    GELU = AF.Gelu if gelu_override is None else gelu_override
    FP32 = mybir.dt.float32
    BF16 = mybir.dt.bfloat16

    nc = bacc.Bacc()

    # bundled inputs: big per-partition rows minimize DMA packet counts
    d_m1 = nc.declare_dram_parameter("m1", [128, 4096], BF16, False)  # wk1|xt0
    d_m2 = nc.declare_dram_parameter("m2", [128, 3072], BF16, False)  # xt1|wk2d|wq2d
    d_b2 = nc.declare_dram_parameter("b2", [128, 2048], BF16, False)  # wq1
    d_b3 = nc.declare_dram_parameter("b3", [128, 4096], BF16, False)  # wv|wg
    d_xnat = nc.declare_dram_parameter("xnat", [128, NSC * D], BF16, False)
    d_cf32 = nc.declare_dram_parameter("cf32", [128, CF_W], FP32, False)
    d_tgr = nc.declare_dram_parameter("tgr", [1, 256], FP32, False)
    d_spidx = nc.declare_dram_parameter("spidx", [128, NSC], FP32, False)
    d_cw = nc.declare_dram_parameter("cw", [1, D], BF16, False)
    d_bvr = nc.declare_dram_parameter("bvr", [1, D], BF16, False)
    d_obr = nc.declare_dram_parameter("obr", [1, D], BF16, False)
    d_out = nc.declare_dram_parameter("out", [128, 2 * D], BF16, True)

    with tile.TileContext(nc) as tc, ExitStack() as ctx:
        consts = ctx.enter_context(tc.tile_pool(name="consts", bufs=1))
        work = ctx.enter_context(tc.tile_pool(name="work", bufs=1))
        atm_pool = ctx.enter_context(tc.tile_pool(name="atm", bufs=4))
        small = ctx.enter_context(tc.tile_pool(name="small", bufs=1))
        # PSUM budget (8 banks): "mlp" 4 + "bank" 1 + "acc" 3
        ps_mlp = ctx.enter_context(tc.tile_pool(name="ps_mlp", bufs=1, space="PSUM"))
        ps_bank = ctx.enter_context(tc.tile_pool(name="ps_bank", bufs=1, space="PSUM"))
        ps_acc = ctx.enter_context(tc.tile_pool(name="ps_acc", bufs=1, space="PSUM"))

        # ---- SBUF const tiles (bundles + views) ----
        mb1 = consts.tile([128, 4096], BF16)
        mb2 = consts.tile([128, 3072], BF16)
        wb3 = consts.tile([128, 4096], BF16)
        xnt = consts.tile([128, NSC * D], BF16)
        xnat = xnt.rearrange("p (c f) -> p c f", c=NSC)   # slot-permuted chunks
        wk1 = mb1[:, 0:2048].rearrange("p (c f) -> p c f", c=4)
        xt0 = mb1[:, 2048:4096].rearrange("p (c f) -> p c f", c=4)  # m0 halves
        xt1 = mb2[:, 0:2048].rearrange("p (c f) -> p c f", c=4)     # m1 halves
        wk2d = mb2[:, 2048:2560].rearrange("p (c f) -> p c f", c=4)
        wq2d = mb2[:, 2560:3072].rearrange("p (c f) -> p c f", c=4)
        wq1 = consts.tile([128, 4, D], BF16)
        wv = wb3[:, 0:2048].rearrange("p (c f) -> p c f", c=4)
        wg = wb3[:, 2048:4096].rearrange("p (c f) -> p c f", c=4)
        cf32 = consts.tile([128, CF_W], FP32)
        tgr = consts.tile([1, 256], FP32)
        spidx = consts.tile([128, NSC], FP32)
        cw = consts.tile([1, D], BF16)
        bvr = consts.tile([1, D], BF16)
        obr = consts.tile([1, D], BF16)
        ones = consts.tile([128, 1], BF16)
        onesr = consts.tile([1, 128], BF16)
        onesr32 = consts.tile([1, 128], FP32)
        cosbias = consts.tile([128, 1], FP32)
        sinscale = consts.tile([128, 1], FP32)

        bk1 = cf32[:, CF_BK1:CF_BK1 + 4]
        bq1 = cf32[:, CF_BQ1:CF_BQ1 + 4]
        bk2d = cf32[:, CF_BK2D:CF_BK2D + 1]
        bq2d = cf32[:, CF_BQ2D:CF_BQ2D + 1]
        epsn2 = cf32[:, CF_EPSN2:CF_EPSN2 + 2]

        # ---- DMAs: three streams, big rows, issued in need order ----
        # sync HWDGE: wk1|xt0 mega-bundle gates the first matmuls
        nc.sync.dma_start(out=mb1, in_=d_m1[:])
        # scalar HWDGE: xt1|wk2d|wq2d, then wv|wg
        nc.scalar.dma_start(out=mb2, in_=d_m2[:])
        nc.scalar.dma_start(out=wb3, in_=d_b3[:])
        # gpsimd SWDGE (fast, independent): biases, wq1, xnat, smalls
        nc.gpsimd.dma_start(out=cf32, in_=d_cf32[:])
        nc.gpsimd.dma_start(out=wq1, in_=d_b2[:])
        nc.gpsimd.dma_start(out=xnt, in_=d_xnat[:])
        nc.gpsimd.dma_start(out=tgr, in_=d_tgr[:])
        nc.gpsimd.dma_start(out=spidx, in_=d_spidx[:])
        nc.gpsimd.dma_start(out=obr, in_=d_obr[:])
        nc.gpsimd.dma_start(out=cw, in_=d_cw[:])
        nc.gpsimd.dma_start(out=bvr, in_=d_bvr[:])

        nc.vector.memset(ones, 1.0)
        nc.vector.memset(onesr, 1.0)
        nc.vector.memset(onesr32, 1.0)
        nc.vector.memset(cosbias[0:64, :], PI / 2)
        nc.vector.memset(cosbias[64:128, :], 0.0)
        nc.vector.memset(sinscale[0:64, :], -PI)
        nc.vector.memset(sinscale[64:128, :], PI)

        # ---- working SBUF tiles ----
        hkT = work.tile([128, 4, L], BF16)
        hqT = work.tile([128, 4, 256], BF16)
        kph2 = work.tile([128, L], BF16)
        qph2 = work.tile([128, 256], BF16)
        KS = work.tile([128, L], BF16)          # rows 0:64 cosK, 64:128 sinK
        QS = work.tile([128, 256], BF16)
        tgb = work.tile([128, 256], FP32)       # tglob broadcast over partitions
        obb = work.tile([128, D], BF16)         # out_bias broadcast
        uT_sb = work.tile([128, 4, 256], BF16)  # (A@x)^T
        arow_sb = work.tile([1, 256], BF16)     # masked row sums of A
        rT_sb = work.tile([128, 4, 256], BF16)  # retrieved^T
        rsq = work.tile([128, 4, 256], BF16)
        out_sb = work.tile([128, 2, D], BF16)

        # ---- key MLP1: (m,j) groups, c-inner, 4-deep psum rotation;
        # program order matches DMA arrival: m0 -> query -> m1 ----
        xtm = [xt0, xt1]

        def key_groups(m):
            for j in range(4):
                ps = ps_mlp.tile([128, D], FP32, tag="mlp")
                for ci in range(4):
                    nc.tensor.matmul(
                        ps,
                        lhsT=wk1[:, ci, j * 128:(j + 1) * 128],
                        rhs=xtm[m][:, ci, :],
                        start=(ci == 0),
                        stop=(ci == 3),
                    )
                nc.scalar.activation(
                    out=hkT[:, j, m * 512:(m + 1) * 512], in_=ps,
                    func=GELU, bias=bk1[:, j:j + 1], scale=1.0,
                )

        def kphase(m):
            ps_k = ps_bank.tile([128, 512], FP32, tag="bank")
            for j in range(4):
                nc.tensor.matmul(
                    ps_k,
                    lhsT=wk2d[:, j, :],
                    rhs=hkT[:, j, m * 512:(m + 1) * 512],
                    start=(j == 0),
                    stop=(j == 3),
                )
            sl = slice(m * 512, (m + 1) * 512)
            nc.scalar.activation(out=kph2[:, sl], in_=ps_k,
                                 func=AF.Tanh, bias=bk2d, scale=1.0)
            nc.vector.scalar_tensor_tensor(
                out=kph2[0:64, sl], in0=kph2[0:64, sl], scalar=-1.0,
                in1=kph2[0:64, sl], op0=ALU.mult, op1=ALU.max)
            nc.scalar.activation(out=KS[:, sl], in_=kph2[:, sl],
                                 func=AF.Sin, bias=cosbias, scale=sinscale)

        key_groups(0)

        # query MLP1 in the acc slot (bank-pair start trick);
        # rhs = own-strip cols (slots 0,1 live in xt0)
        qps = ps_acc.tile([128, 6, 256], FP32, tag="acc")
        for ci in range(4):
            for j in range(4):
                nc.tensor.matmul(
                    qps[:, j, :],
                    lhsT=wq1[:, ci, j * 128:(j + 1) * 128],
                    rhs=xt0[:, ci, 0:256],
                    start=(ci == 0 and j in (0, 2)),
                    stop=(ci == 3 and j in (1, 3)),
                )
        for j in range(4):
            nc.scalar.activation(
                out=hqT[:, j, :], in_=qps[:, j, :],
                func=GELU, bias=bq1[:, j:j + 1], scale=1.0,
            )

        kphase(0)

        # qphase: tanh -> |t| -> sin
        ps_p = ps_bank.tile([128, 512], FP32, tag="bank")
        for j in range(4):
            nc.tensor.matmul(
                ps_p[:, 0:256],
                lhsT=wq2d[:, j, :],
                rhs=hqT[:, j, :],
                start=(j == 0),
                stop=(j == 3),
            )
        nc.scalar.activation(out=qph2, in_=ps_p[:, 0:256], func=AF.Tanh,
                             bias=bq2d, scale=1.0)
        nc.vector.scalar_tensor_tensor(
            out=qph2[0:64, :], in0=qph2[0:64, :], scalar=-1.0,
            in1=qph2[0:64, :], op0=ALU.mult, op1=ALU.max)
        nc.scalar.activation(out=QS, in_=qph2, func=AF.Sin,
                             bias=cosbias, scale=sinscale)

        key_groups(1)
        kphase(1)

        # prefetch the sqrt ACT table off the critical tail (dep on sin m1
        # keeps the scheduler from hoisting it before the gelu table load)
        sqd = small.tile([1, 1], FP32)
        nc.scalar.activation(out=sqd, in_=KS[0:1, 1023:1024], func=AF.Sqrt)

        # ---- broadcasts (PE rank-1): tglob over partitions; out_bias ----
        ps_t = ps_bank.tile([128, 512], FP32, tag="bank")
        nc.tensor.matmul(ps_t[:, 0:256], lhsT=onesr32, rhs=tgr,
                         start=True, stop=True)
        nc.vector.tensor_scalar_mul(out=tgb, in0=ps_t[:, 0:256], scalar1=1.0)
        ps_o = ps_bank.tile([128, 512], FP32, tag="bank")
        nc.tensor.matmul(ps_o, lhsT=onesr, rhs=obr, start=True, stop=True)
        nc.vector.tensor_scalar_mul(out=obb, in0=ps_o, scalar1=1.0)

        # ---- scores + on-device causal mask + uT/arow accumulation ----
        # acc slot: dc accumulators in banks 0-1 (chunks 0-3), arow in bank 2
        utp = ps_acc.tile([128, 6, 256], FP32, tag="acc")
        ut_ps = utp[:, 0:4, :]
        arow_ps = utp[0:1, 4, :]
        # all 8 score tiles live at once in the retired key-MLP psum slots
        # (4 banks); even slots carry each bank's start (zeroing both halves)
        atps = []
        for k in range(4):
            atp = ps_mlp.tile([128, 2, 256], FP32, tag="mlp")
            atps.append(atp)
        for sc in range(NSC):
            nc.tensor.matmul(
                atps[sc // 2][:, sc % 2, :],
                lhsT=KS[:, sc * 128:(sc + 1) * 128],
                rhs=QS,
                start=(sc % 2 == 0),
                stop=(sc % 2 == 1),
            )
        for sc in range(NSC):
            atm = atm_pool.tile([128, 256], BF16, tag="atm")
            # atm[p,t] = (tglob[t] >= s_glob[p]) * at[p,t]
            nc.vector.scalar_tensor_tensor(
                out=atm, in0=tgb, scalar=spidx[:, sc:sc + 1],
                in1=atps[sc // 2][:, sc % 2, :], op0=ALU.is_ge, op1=ALU.mult)
            for dc in range(NDC):
                nc.tensor.matmul(
                    ut_ps[:, dc, :],
                    lhsT=xnat[:, sc, dc * 128:(dc + 1) * 128],
                    rhs=atm,
                    start=(sc == 0 and dc in (0, 2)),
                    stop=(sc == NSC - 1 and dc in (1, 3)),
                )
            nc.tensor.matmul(
                arow_ps,
                lhsT=ones,
                rhs=atm,
                start=(sc == 0),
                stop=(sc == NSC - 1),
            )

        # ---- uT -> SBUF bf16; retrievedT = Wv^T @ uT + bv (x) arow ----
        for ci in range(4):
            if ci % 2 == 0:
                nc.vector.tensor_scalar_mul(out=uT_sb[:, ci, :],
                                            in0=ut_ps[:, ci, :], scalar1=1.0)
            else:
                nc.scalar.copy(out=uT_sb[:, ci, :], in_=ut_ps[:, ci, :])
        nc.vector.tensor_scalar_mul(out=arow_sb, in0=arow_ps, scalar1=1.0)

        rtp = ps_acc.tile([128, 6, 256], FP32, tag="acc")
        rt_ps = rtp[:, 0:4, :]
        for dc in range(NDC):
            for ci in range(4):
                nc.tensor.matmul(
                    rt_ps[:, dc, :],
                    lhsT=wv[:, ci, dc * 128:(dc + 1) * 128],
                    rhs=uT_sb[:, ci, :],
                    start=(dc in (0, 2) and ci == 0),
                    stop=False,
                )
            nc.tensor.matmul(
                rt_ps[:, dc, :],
                lhsT=bvr[:, dc * 128:(dc + 1) * 128],
                rhs=arow_sb,
                start=False,
                stop=(dc in (1, 3)),
            )

        for dc in range(NDC):
            if dc % 2 == 0:
                nc.vector.tensor_scalar_mul(out=rT_sb[:, dc, :],
                                            in0=rt_ps[:, dc, :], scalar1=1.0)
            else:
                nc.scalar.copy(out=rT_sb[:, dc, :], in_=rt_ps[:, dc, :])
        for dc in range(NDC):
            nc.vector.tensor_mul(out=rsq[:, dc, :], in0=rT_sb[:, dc, :],
                                 in1=rT_sb[:, dc, :])

        # ---- row stats (matmul partition reductions) ----
        # one acc slot: sums in bank 0, row sums in bank 2 (disjoint starts)
        stp = ps_acc.tile([128, 6, 256], FP32, tag="acc")
        sums_ps = stp[:, 0, 0:4]
        row_ps = stp[0:1, 4, :]
        first = True
        n = 0
        for st in range(2):
            for src, col in ((rT_sb, st), (rsq, 2 + st)):
                for dc in range(NDC):
                    n += 1
                    nc.tensor.matmul(
                        sums_ps[:, col:col + 1],
                        lhsT=src[:, dc, st * 128:(st + 1) * 128],
                        rhs=ones,
                        start=first,
                        stop=(n == 16),
                    )
                    first = False
        for dc in range(NDC):
            nc.tensor.matmul(
                row_ps,
                lhsT=ones,
                rhs=rT_sb[:, dc, :],
                start=(dc == 0),
                stop=(dc == 3),
            )

        negmu = small.tile([1, 256], BF16)
        nc.vector.tensor_scalar_mul(out=negmu, in0=row_ps, scalar1=-1.0 / D)

        mu = small.tile([128, 2], FP32)
        musq = small.tile([128, 2], FP32)
        var = small.tile([128, 2], FP32)
        scl = small.tile([128, 2], FP32)
        for st in range(2):
            nc.vector.tensor_scalar_mul(out=mu[:, st:st + 1],
                                        in0=sums_ps[:, st:st + 1],
                                        scalar1=1.0 / D)
            nc.vector.tensor_mul(out=musq[:, st:st + 1],
                                 in0=mu[:, st:st + 1], in1=mu[:, st:st + 1])
            nc.vector.scalar_tensor_tensor(
                out=var[:, st:st + 1],
                in0=sums_ps[:, 2 + st:3 + st],
                scalar=1.0 / D,
                in1=musq[:, st:st + 1],
                op0=ALU.mult,
                op1=ALU.subtract,
            )
        for st in range(2):
            nc.scalar.activation(out=scl[:, st:st + 1], in_=var[:, st:st + 1],
                                 func=AF.Sqrt, bias=epsn2[:, st:st + 1],
                                 scale=1.0)
            nc.vector.reciprocal(out=scl[:, st:st + 1], in_=scl[:, st:st + 1])

        # ---- output: out = scl*(rT^T@Wg - mu*cw) + x + out_bias ----
        for st in range(2):
            reg = ps_bank.tile([128, D], FP32, tag="bank")
            for dc in range(NDC):
                nc.tensor.matmul(
                    reg,
                    lhsT=rT_sb[:, dc, st * 128:(st + 1) * 128],
                    rhs=wg[:, dc, :],
                    start=(dc == 0),
                    stop=False,
                )
            nc.tensor.matmul(
                reg,
                lhsT=negmu[:, st * 128:(st + 1) * 128],
                rhs=cw,
                start=False,
                stop=True,
            )
            nc.vector.scalar_tensor_tensor(
                out=out_sb[:, st, :],
                in0=reg,
                scalar=scl[:, st:st + 1],
                in1=xnat[:, st, :],
                op0=ALU.mult,
                op1=ALU.add,
            )
            nc.vector.tensor_add(out=out_sb[:, st, :], in0=out_sb[:, st, :],
                                 in1=obb)
            if st == 0:
                nc.sync.dma_start(out=d_out[:, 0:D], in_=out_sb[:, 0, :])
            else:
                nc.scalar.dma_start(out=d_out[:, D:2 * D], in_=out_sb[:, 1, :])

    return nc


def _host_prepare(inputs):
    """Build the 8 per-core input maps (host-side numpy packing)."""
    import ml_dtypes

    bf16 = ml_dtypes.bfloat16
    f32 = np.float32

    x = np.asarray(inputs["x"], f32)
    Wk1 = np.asarray(inputs["Wk1"], f32)
    bk1 = np.asarray(inputs["bk1"], f32)
    Wk2 = np.asarray(inputs["Wk2"], f32)
    bk2 = np.asarray(inputs["bk2"], f32)
    Wq1 = np.asarray(inputs["Wq1"], f32)
    bq1 = np.asarray(inputs["bq1"], f32)
    Wq2 = np.asarray(inputs["Wq2"], f32)
    bq2 = np.asarray(inputs["bq2"], f32)
    Wv = np.asarray(inputs["Wv"], f32)
    bv = np.asarray(inputs["bv"], f32)
    ln_g = np.asarray(inputs["ln_g"], f32)
    ln_b = np.asarray(inputs["ln_b"], f32)
    Wo = np.asarray(inputs["Wo"], f32)
    bo = np.asarray(inputs["bo"], f32)

    Wg32 = ln_g[:, None] * Wo
    cw = Wg32.astype(bf16).astype(f32).sum(axis=0).astype(bf16).reshape(1, D)
    out_bias = (ln_b @ Wo + bo).astype(f32).reshape(1, D)

    def pack(w):  # [D_in, F] -> [128, 4, F] -> [128, 4F]
        return np.ascontiguousarray(
            w.reshape(4, 128, -1).transpose(1, 0, 2)).reshape(128, -1)

    wk2d = np.concatenate([Wk2, Wk2], axis=1)  # [512, 128]
    wq2d = np.concatenate([Wq2, Wq2], axis=1)

    b3 = np.concatenate([pack(Wv), pack(Wg32)], axis=1).astype(bf16)
    base = {
        "b2": pack(Wq1).astype(bf16),
        "b3": b3,
        "cw": cw,
        "bvr": bv.reshape(1, D).astype(bf16),
        "obr": out_bias.astype(bf16),
    }
    wk1p = pack(Wk1)
    w2p = np.concatenate([pack(wk2d), pack(wq2d)], axis=1)

    in_maps = []
    for c in range(NCORES):
        b, i = divmod(c, 4)
        own = [i, 7 - i]
        perm = own + [s for s in range(NSC) if s not in own]
        rowidx = np.concatenate(
            [np.arange(p * 128, (p + 1) * 128) for p in perm])
        xb = x[b]  # [L, D]
        xTp = pack(np.ascontiguousarray(xb.T[:, rowidx]))  # [128, 4*L]
        xnat = np.ascontiguousarray(
            xb[rowidx].reshape(NSC, 128, D).transpose(1, 0, 2)).reshape(128, -1)
        spidx = (np.array(perm, f32)[None, :] * 128.0
                 + np.arange(128, dtype=f32)[:, None])
        tglob = np.concatenate(
            [np.arange(i * 128, (i + 1) * 128),
             np.arange((7 - i) * 128, (8 - i) * 128)]).astype(f32)
        epsn2 = (EPS * K * (tglob + 1.0)).reshape(2, 128).T  # [128, 2]

        cf = np.zeros((128, CF_W), f32)
        cf[:, CF_BK1:CF_BK1 + 4] = bk1.reshape(4, 128).T
        cf[:, CF_BQ1:CF_BQ1 + 4] = bq1.reshape(4, 128).T
        cf[:, CF_BK2D] = np.concatenate([bk2, bk2])
        cf[:, CF_BQ2D] = np.concatenate([bq2, bq2])
        cf[:, CF_EPSN2:CF_EPSN2 + 2] = epsn2

        xTc = xTp.reshape(128, 4, L)
        xt0 = xTc[:, :, 0:512].reshape(128, 2048)
        xt1 = xTc[:, :, 512:1024].reshape(128, 2048)
        m = dict(base)
        m["m1"] = np.concatenate([wk1p, xt0], axis=1).astype(bf16)
        m["m2"] = np.concatenate([xt1, w2p], axis=1).astype(bf16)
        m["xnat"] = xnat.astype(bf16)
        m["cf32"] = cf
        m["tgr"] = tglob.reshape(1, 256)
        m["spidx"] = spidx.astype(f32)
        in_maps.append(m)
    return in_maps


def run(inputs, trace=False):
    from concourse.bass_utils import run_bass_kernel_spmd

    if "nc" not in _CACHE:
        nc = _build_program()
        nc.finalize()
        _CACHE["nc"] = nc
    nc = _CACHE["nc"]
    in_maps = _host_prepare(inputs)
    res = run_bass_kernel_spmd(nc, in_maps, list(range(NCORES)), trace=trace)
    out = np.empty((B, L, D), np.float32)
    for c in range(NCORES):
        b, i = divmod(c, 4)
        oc = np.asarray(res.results[c]["out"], np.float32)  # [128, 2*D]
        out[b, i * 128:(i + 1) * 128] = oc[:, 0:D]
        out[b, (7 - i) * 128:(8 - i) * 128] = oc[:, D:2 * D]
    return out, res


def kernel(**inputs):
    out, _ = run(inputs, trace=False)
    return out
